# revision 12
# baseline (speedup 1.0000x reference)
"""CP(n) lattice action kernel for Trainium2 (8 NeuronCores, Bass/Tile).

Math (matches reference):
  phi: (B=1024, S=4096, n=6) angles; shift: (2, S) neighbor site indices.
  Wrap: first 5 angles mod pi, last mod 2pi.
  x = hyperspherical embedding (7 comps); z = (x0..x3) + i(x4,x5,x6,0).
  w_d(s) = sum_k z_k(s) z_k(shift[d,s])
  action[b] = -4 * sum_{d,s} (|w_d(s)|^2 - 1)

Implementation notes:
  - Pure data parallel: batch axis sharded 8 x 128; batch on SBUF partitions,
    sites along the free dimension.
  - This platform is per-instruction-overhead dominated, so the kernel is
    built from few, large, multi-row instructions.
  - Angle wrap via identities (no floating mod on TRN2):
      phir = phi - 2pi*round(phi/2pi)  (round via fp->int RNE convert)
      sigma = Sign(phir)
      s_j  = sigma*Sin(phir) (j<5);  s_5 = Sin(phir)
      c_j  = Sin(sigma*pi/2 - phir) = sigma*cos(phir) (j<5); c_5 = sigma*that
  - Neighbor gather: shift values are read on host at build time; the
    roll-structured shift (nearest neighbor on a 64x64 lattice) lowers to
    offset/strided access patterns; arbitrary shift falls back to per-run
    copies.
  - Per-site |w|^2 and the site reduction fuse into ACT Square + accum_out.
"""
import contextlib
import sys

import numpy as np

sys.path.insert(0, "/opt/trn_rl_repo")

B, S, NA = 1024, 4096, 6
NCORES = 8
PB = B // NCORES          # 128 batches per core
C1 = 1024                 # stage-1 site chunk
N1 = S // C1
C2 = 1024                 # stage-2 site chunk
N2 = S // C2
L = 64                    # lattice row length
PI = float(np.pi)
NBETA = 4.0               # N * BETA

_cache = {}


def _detect_roll(shift):
    idx = np.arange(S).reshape(L, L)
    s0 = np.roll(idx, -1, axis=0).ravel()
    s1 = np.roll(idx, -1, axis=1).ravel()
    return np.array_equal(shift[0], s0) and np.array_equal(shift[1], s1)


def _runs(perm):
    runs = []
    st = 0
    for i in range(1, len(perm) + 1):
        if i == len(perm) or perm[i] != perm[i - 1] + 1:
            runs.append((st, int(perm[st]), i - st))
            st = i
    return runs


def _build(shift, reps=1, mode="full"):
    import concourse.bass as bass
    import concourse.tile as tile
    from concourse import bacc, mybir

    f32 = mybir.dt.float32
    bf16 = mybir.dt.bfloat16
    i16 = mybir.dt.int16
    Act = mybir.ActivationFunctionType
    Op = mybir.AluOpType
    X = mybir.AxisListType.X

    roll = _detect_roll(shift)

    nc = bacc.Bacc(None, target_bir_lowering=False)
    phi_d = nc.dram_tensor("phi", [PB, S, NA], f32, kind="ExternalInput")
    out_d = nc.dram_tensor("out", [PB, 1], f32, kind="ExternalOutput")
    pd_flat = phi_d[:].rearrange("p s a -> p (s a)")

    with tile.TileContext(nc) as tc:
        with contextlib.ExitStack() as ctx:
            xfull_pool = ctx.enter_context(tc.tile_pool(name="xfull", bufs=1))
            small_pool = ctx.enter_context(tc.tile_pool(name="small", bufs=1))

            NACC = N2 * 2
            acc = small_pool.tile([PB, NACC], f32)
            if roll:
                xf = xfull_pool.tile([PB, 7, S], bf16)
                xg = None
            else:
                # site-major cells [site, 8] so gpsimd.ap_gather can fetch
                # whole 7-component cells per shift index
                xf = None
                xg = xfull_pool.tile([PB, S, 8], bf16)
                idx_sb = []
                for d in range(2):
                    wrapped = np.zeros((PB, S // 16), np.int16)
                    base = shift[d].reshape(S // 16, 16).T.astype(np.int16)
                    for g in range(PB // 16):
                        wrapped[16 * g:16 * (g + 1)] = base
                    hdl = nc.inline_tensor(wrapped, name=f"shift_idx_{d}")
                    t_ = small_pool.tile([PB, S // 16], mybir.dt.int16, tag=f"idx{d}")
                    nc.sync.dma_start(t_[:], hdl[:])
                    idx_sb.append(t_)

            for rep in range(reps):
                # ======== stage 1: wrap + trig + embedding ========
                st1 = contextlib.ExitStack()
                p_phi = st1.enter_context(tc.tile_pool(name="p_phi", bufs=1))
                p_ks = st1.enter_context(tc.tile_pool(name="p_ks", bufs=1))
                p_t = st1.enter_context(tc.tile_pool(name="p_t", bufs=1))
                p_u = st1.enter_context(tc.tile_pool(name="p_u", bufs=1))
                p_cum = st1.enter_context(tc.tile_pool(name="p_cum", bufs=1))
                p_xp = st1.enter_context(tc.tile_pool(name="p_xp", bufs=2))
                p_m = st1.enter_context(tc.tile_pool(name="p_m", bufs=2))
                p_pq = st1.enter_context(
                    tc.tile_pool(name="p_pq", bufs=2 if roll else 1))
                junk_pool = st1.enter_context(tc.tile_pool(name="junk", bufs=1))

                for ch in range(N1):
                    cs = ch * C1
                    M = C1 * NA

                    phic = p_phi.tile([PB, M], f32, tag="phic")
                    nc.sync.dma_start(phic[:], pd_flat[:, cs * NA:(cs + C1) * NA])

                    if mode == "dma":
                        nc.vector.tensor_reduce(acc[:, 0:1], phic[:, 0:8],
                                                axis=X, op=Op.add)
                        continue

                    # k = round(phi/2pi) as int16
                    k = p_ks.tile([PB, M], i16, tag="ks")
                    nc.vector.tensor_scalar(k[:], phic[:], 1.0 / (2 * PI), None,
                                            op0=Op.mult)
                    # phir = (k * -2pi) + phi   (in place)
                    nc.vector.scalar_tensor_tensor(
                        phic[:], k[:], -2 * PI, phic[:], op0=Op.mult, op1=Op.add)

                    # sigma, t = Sin(phir)  (interleaved site-major, bf16)
                    sig = p_ks.tile([PB, M], bf16, tag="ks")
                    nc.scalar.activation(sig[:], phic[:], Act.Sign)
                    tt = p_t.tile([PB, M], bf16, tag="t")
                    nc.scalar.activation(tt[:], phic[:], Act.Sin)
                    # arg2 = sigma*pi/2 - phir (in place over phir)
                    nc.vector.scalar_tensor_tensor(
                        phic[:], sig[:], PI / 2, phic[:],
                        op0=Op.mult, op1=Op.subtract)
                    # u = Sin(arg2) = sigma*cos(phir)
                    uu = p_u.tile([PB, M], bf16, tag="u")
                    nc.scalar.activation(uu[:], phic[:], Act.Sin)

                    def ang(tile_, j, n=1):
                        ap = tile_[:]
                        if n == 1:
                            return bass.AP(tensor=ap.tensor, offset=ap.offset + j,
                                           ap=[ap.ap[0], [NA, C1]])
                        return bass.AP(tensor=ap.tensor, offset=ap.offset + j,
                                       ap=[ap.ap[0], [NA, C1], [1, n]])

                    # s_j = sigma*t for j<5 (in place on t)
                    nc.vector.tensor_tensor(ang(tt, 0, 5), ang(tt, 0, 5),
                                            ang(sig, 0, 5), op=Op.mult)
                    # c_5 = sigma*u at j=5 (in place on u)
                    nc.vector.tensor_tensor(ang(uu, 5), ang(uu, 5),
                                            ang(sig, 5), op=Op.mult)

                    # cumprod + x build into xf rows / xg cells
                    cumA = p_cum.tile([PB, C1], bf16, tag="cumA")
                    cumB = p_cum.tile([PB, C1], bf16, tag="cumB")
                    if roll:
                        xs = xf[:, :, cs:cs + C1]
                        xk = [xs[:, k, :] for k in range(7)]
                    else:
                        gap = xg[:]
                        xk = [bass.AP(tensor=gap.tensor,
                                      offset=gap.offset + cs * 8 + k,
                                      ap=[gap.ap[0], [8, C1]])
                              for k in range(7)]
                    nc.vector.tensor_copy(xk[0], ang(uu, 0))
                    nc.vector.tensor_tensor(xk[1], ang(uu, 1), ang(tt, 0),
                                            op=Op.mult)
                    nc.vector.tensor_tensor(cumA[:], ang(tt, 0), ang(tt, 1),
                                            op=Op.mult)
                    nc.vector.tensor_tensor(xk[2], ang(uu, 2), cumA[:],
                                            op=Op.mult)
                    nc.vector.tensor_tensor(cumB[:], cumA[:], ang(tt, 2),
                                            op=Op.mult)
                    nc.vector.tensor_tensor(xk[3], ang(uu, 3), cumB[:],
                                            op=Op.mult)
                    nc.vector.tensor_tensor(cumA[:], cumB[:], ang(tt, 3),
                                            op=Op.mult)
                    nc.vector.tensor_tensor(xk[4], ang(uu, 4), cumA[:],
                                            op=Op.mult)
                    nc.vector.tensor_tensor(cumB[:], cumA[:], ang(tt, 4),
                                            op=Op.mult)
                    nc.vector.tensor_tensor(xk[5], ang(uu, 5), cumB[:],
                                            op=Op.mult)
                    nc.vector.tensor_tensor(xk[6], cumB[:], ang(tt, 5),
                                            op=Op.mult)

                if mode in ("dma", "stage1"):
                    st1.close()
                    continue

                # ======== stage 2: neighbor products (pools shared) ========

                for ch in range(N2):
                    cs = ch * C2
                    if roll:
                        xs = xf[:, :, cs:cs + C2]
                    else:
                        gap = xg[:]
                        xs = None
                        xg_k = lambda k0, n, off=0: bass.AP(
                            tensor=gap.tensor,
                            offset=gap.offset + cs * 8 + k0,
                            ap=[gap.ap[0], [1, n], [8, C2]])

                    for d in (0, 1):
                        # x' view(s) for this dir
                        if roll and d == 0:
                            lo = cs + L
                            if lo + C2 <= S:
                                xp_ap = xf[:, :, lo:lo + C2]
                            else:
                                xp = p_xp.tile([PB, 7, C2], bf16, tag="xp")
                                mn = S - lo
                                nc.vector.tensor_copy(xp[:, :, 0:mn],
                                                      xf[:, :, lo:S])
                                nc.vector.tensor_copy(xp[:, :, mn:C2],
                                                      xf[:, :, 0:C2 - mn])
                                xp_ap = xp[:]
                        elif roll and d == 1:
                            xp = p_xp.tile([PB, 7, C2], bf16, tag="xp")
                            nrow = C2 // L
                            src = bass.AP(
                                tensor=xf.tensor, offset=xf[:].offset + cs + 1,
                                ap=[xf[:].ap[0], [S, 7], [L, nrow], [1, L - 1]])
                            dst = bass.AP(
                                tensor=xp.tensor, offset=xp[:].offset,
                                ap=[xp[:].ap[0], [C2, 7], [L, nrow], [1, L - 1]])
                            nc.gpsimd.tensor_copy(dst, src)
                            srcw = bass.AP(
                                tensor=xf.tensor, offset=xf[:].offset + cs,
                                ap=[xf[:].ap[0], [S, 7], [L, nrow]])
                            dstw = bass.AP(
                                tensor=xp.tensor, offset=xp[:].offset + L - 1,
                                ap=[xp[:].ap[0], [C2, 7], [L, nrow]])
                            nc.gpsimd.tensor_copy(dstw, srcw)
                            xp_ap = xp[:]
                        else:
                            xpg = p_xp.tile([PB, C2, 8], bf16, tag="xp")
                            nc.gpsimd.ap_gather(
                                xpg[:], xg[:],
                                idx_sb[d][:, cs // 16:(cs + C2) // 16],
                                channels=PB, num_elems=S, d=8, num_idxs=C2)
                            gp = xpg[:]
                            xp_k = lambda k0, n: bass.AP(
                                tensor=gp.tensor, offset=gp.offset + k0,
                                ap=[gp.ap[0], [1, n], [8, C2]])

                        # m_k = x_k * x'_k  (7 rows, one op)
                        m = p_m.tile([PB, 7, C2], bf16, tag="m")
                        pq = p_pq.tile([PB, 6, C2], bf16, tag="pq")
                        if roll:
                            nc.vector.tensor_tensor(m[:], xs, xp_ap, op=Op.mult)
                            nc.vector.tensor_tensor(
                                pq[:, 0:3, :], xs[:, 0:3, :],
                                xp_ap[:, 4:7, :], op=Op.mult)
                            nc.vector.tensor_tensor(
                                pq[:, 3:6, :], xs[:, 4:7, :],
                                xp_ap[:, 0:3, :], op=Op.mult)
                        else:
                            nc.vector.tensor_tensor(m[:], xg_k(0, 7),
                                                    xp_k(0, 7), op=Op.mult)
                            nc.vector.tensor_tensor(pq[:, 0:3, :], xg_k(0, 3),
                                                    xp_k(4, 3), op=Op.mult)
                            nc.vector.tensor_tensor(pq[:, 3:6, :], xg_k(4, 3),
                                                    xp_k(0, 3), op=Op.mult)

                        # wr = m0+m1+m2+m3-m4-m5-m6 via subtract-fold, into m[0]
                        nc.vector.tensor_tensor(m[:, 0:3, :], m[:, 0:3, :],
                                                m[:, 4:7, :], op=Op.subtract)
                        nc.vector.tensor_tensor(m[:, 0:2, :], m[:, 0:2, :],
                                                m[:, 2:4, :], op=Op.add)
                        nc.vector.tensor_tensor(m[:, 0, :], m[:, 0, :],
                                                m[:, 1, :], op=Op.add)
                        # wi = sum(pq rows), into m[1]
                        nc.vector.tensor_tensor(pq[:, 0:3, :], pq[:, 0:3, :],
                                                pq[:, 3:6, :], op=Op.add)
                        nc.vector.tensor_tensor(pq[:, 0, :], pq[:, 0, :],
                                                pq[:, 1, :], op=Op.add)
                        nc.vector.tensor_tensor(m[:, 1, :], pq[:, 0, :],
                                                pq[:, 2, :], op=Op.add)

                        # acc += sum_s wr^2 + wi^2, one fused square over 2 rows
                        ia = ch * 2 + d
                        j1 = junk_pool.tile([PB, 2, C2], bf16, tag="junk")
                        nc.scalar.activation(j1[:], m[:, 0:2, :], Act.Square,
                                             accum_out=acc[:, ia:ia + 1])

                st1.close()

            # ======== final reduce + affine ========
            stot = small_pool.tile([PB, 1], f32)
            nc.vector.tensor_reduce(stot[:], acc[:], axis=X, op=Op.add)
            res = small_pool.tile([PB, 1], f32)
            nc.vector.tensor_scalar(res[:], stot[:], -NBETA, NBETA * 2.0 * S,
                                    op0=Op.mult, op1=Op.add)
            nc.sync.dma_start(out_d[:], res[:])

    nc.finalize()
    return nc


def kernel(phi, shift):
    from concourse.bass_utils import run_bass_kernel_spmd

    phi = np.ascontiguousarray(np.asarray(phi, dtype=np.float32))
    shift = np.asarray(shift, dtype=np.int32)
    key = (shift.tobytes(), 1)
    if key not in _cache:
        _cache[key] = _build(shift)
    nc = _cache[key]

    in_maps = [{"phi": phi[i * PB:(i + 1) * PB]} for i in range(NCORES)]
    res = run_bass_kernel_spmd(nc, in_maps, core_ids=list(range(NCORES)))
    out = np.concatenate([r["out"] for r in res.results], axis=0)
    return out.astype(np.float32)


# revision 13
# speedup vs baseline: 1.8778x; 1.8778x over previous
"""CP(n) lattice action kernel for Trainium2 (8 NeuronCores, Bass/Tile).

Math (matches reference):
  phi: (B=1024, S=4096, n=6) angles; shift: (2, S) neighbor site indices.
  Wrap: first 5 angles mod pi, last mod 2pi.
  x = hyperspherical embedding (7 comps); z = (x0..x3) + i(x4,x5,x6,0).
  w_d(s) = sum_k z_k(s) z_k(shift[d,s])
  action[b] = -4 * sum_{d,s} (|w_d(s)|^2 - 1)

Implementation notes:
  - Pure data parallel: batch axis sharded 8 x 128; batch on SBUF partitions,
    sites along the free dimension.
  - This platform is per-instruction-overhead dominated, so the kernel is
    built from few, large, multi-row instructions.
  - Angle wrap via identities (no floating mod on TRN2):
      phir = phi - 2pi*round(phi/2pi)  (round via fp->int RNE convert)
      sigma = Sign(phir)
      s_j  = sigma*Sin(phir) (j<5);  s_5 = Sin(phir)
      c_j  = Sin(sigma*pi/2 - phir) = sigma*cos(phir) (j<5); c_5 = sigma*that
  - Neighbor gather: shift values are read on host at build time; the
    roll-structured shift (nearest neighbor on a 64x64 lattice) lowers to
    offset/strided access patterns; arbitrary shift falls back to per-run
    copies.
  - Per-site |w|^2 and the site reduction fuse into ACT Square + accum_out.
"""
import contextlib
import sys

import numpy as np

sys.path.insert(0, "/opt/trn_rl_repo")

B, S, NA = 1024, 4096, 6
NCORES = 8
PB = B // NCORES          # 128 batches per core
C1 = 2048                 # stage-1 site chunk
N1 = S // C1
C2 = 2048                 # stage-2 site chunk
N2 = S // C2
L = 64                    # lattice row length
PI = float(np.pi)
NBETA = 4.0               # N * BETA

_cache = {}


def _detect_roll(shift):
    idx = np.arange(S).reshape(L, L)
    s0 = np.roll(idx, -1, axis=0).ravel()
    s1 = np.roll(idx, -1, axis=1).ravel()
    return np.array_equal(shift[0], s0) and np.array_equal(shift[1], s1)


def _runs(perm):
    runs = []
    st = 0
    for i in range(1, len(perm) + 1):
        if i == len(perm) or perm[i] != perm[i - 1] + 1:
            runs.append((st, int(perm[st]), i - st))
            st = i
    return runs


def _build(shift, reps=1, mode="full"):
    import concourse.bass as bass
    import concourse.tile as tile
    from concourse import bacc, mybir

    f32 = mybir.dt.float32
    bf16 = mybir.dt.bfloat16
    i16 = mybir.dt.int16
    Act = mybir.ActivationFunctionType
    Op = mybir.AluOpType
    X = mybir.AxisListType.X

    roll = _detect_roll(shift)

    nc = bacc.Bacc(None, target_bir_lowering=False)
    phi_d = nc.dram_tensor("phi", [PB, S, NA], f32, kind="ExternalInput")
    out_d = nc.dram_tensor("out", [PB, 1], f32, kind="ExternalOutput")
    pd_flat = phi_d[:].rearrange("p s a -> p (s a)")

    with tile.TileContext(nc) as tc:
        with contextlib.ExitStack() as ctx:
            xfull_pool = ctx.enter_context(tc.tile_pool(name="xfull", bufs=1))
            small_pool = ctx.enter_context(tc.tile_pool(name="small", bufs=1))

            NACC = N2 * 2
            acc = small_pool.tile([PB, NACC], f32)
            if roll:
                xf = xfull_pool.tile([PB, 7, S], bf16)
                xg = None
            else:
                # site-major cells [site, 8] so gpsimd.ap_gather can fetch
                # whole 7-component cells per shift index
                xf = None
                xg = xfull_pool.tile([PB, S, 8], bf16)
                idx_sb = []
                for d in range(2):
                    wrapped = np.zeros((PB, S // 16), np.int16)
                    base = shift[d].reshape(S // 16, 16).T.astype(np.int16)
                    for g in range(PB // 16):
                        wrapped[16 * g:16 * (g + 1)] = base
                    hdl = nc.inline_tensor(wrapped, name=f"shift_idx_{d}")
                    t_ = small_pool.tile([PB, S // 16], mybir.dt.int16, tag=f"idx{d}")
                    nc.sync.dma_start(t_[:], hdl[:])
                    idx_sb.append(t_)

            for rep in range(reps):
                # ======== stage 1: wrap + trig + embedding ========
                st1 = contextlib.ExitStack()
                p_phi = st1.enter_context(tc.tile_pool(name="p_phi", bufs=1))
                p_ks = st1.enter_context(tc.tile_pool(name="p_ks", bufs=1))
                p_t = st1.enter_context(tc.tile_pool(name="p_t", bufs=1))
                p_u = st1.enter_context(tc.tile_pool(name="p_u", bufs=1))
                p_cum = st1.enter_context(tc.tile_pool(name="p_cum", bufs=1))

                for ch in range(N1):
                    cs = ch * C1
                    M = C1 * NA

                    phic = p_phi.tile([PB, M], f32, tag="phic")
                    nc.sync.dma_start(phic[:], pd_flat[:, cs * NA:(cs + C1) * NA])

                    if mode == "dma":
                        nc.vector.tensor_reduce(acc[:, 0:1], phic[:, 0:8],
                                                axis=X, op=Op.add)
                        continue

                    # k = round(phi/2pi) as int16
                    k = p_ks.tile([PB, M], i16, tag="ks")
                    nc.vector.tensor_scalar(k[:], phic[:], 1.0 / (2 * PI), None,
                                            op0=Op.mult)
                    # phir = (k * -2pi) + phi   (in place)
                    nc.vector.scalar_tensor_tensor(
                        phic[:], k[:], -2 * PI, phic[:], op0=Op.mult, op1=Op.add)

                    # sigma, t = Sin(phir)  (interleaved site-major, bf16)
                    sig = p_ks.tile([PB, M], bf16, tag="ks")
                    nc.scalar.activation(sig[:], phic[:], Act.Sign)
                    tt = p_t.tile([PB, M], bf16, tag="t")
                    nc.scalar.activation(tt[:], phic[:], Act.Sin)
                    # arg2 = sigma*pi/2 - phir (in place over phir)
                    nc.vector.scalar_tensor_tensor(
                        phic[:], sig[:], PI / 2, phic[:],
                        op0=Op.mult, op1=Op.subtract)
                    # u = Sin(arg2) = sigma*cos(phir)
                    uu = p_u.tile([PB, M], bf16, tag="u")
                    nc.scalar.activation(uu[:], phic[:], Act.Sin)

                    def ang(tile_, j, n=1):
                        ap = tile_[:]
                        if n == 1:
                            return bass.AP(tensor=ap.tensor, offset=ap.offset + j,
                                           ap=[ap.ap[0], [NA, C1]])
                        return bass.AP(tensor=ap.tensor, offset=ap.offset + j,
                                       ap=[ap.ap[0], [NA, C1], [1, n]])

                    # s_j = sigma*t for j<5 (in place on t)
                    nc.vector.tensor_tensor(ang(tt, 0, 5), ang(tt, 0, 5),
                                            ang(sig, 0, 5), op=Op.mult)
                    # c_5 = sigma*u at j=5 (in place on u)
                    nc.vector.tensor_tensor(ang(uu, 5), ang(uu, 5),
                                            ang(sig, 5), op=Op.mult)

                    # cumprod + x build into xf rows / xg cells
                    cumA = p_cum.tile([PB, C1], bf16, tag="cumA")
                    cumB = p_cum.tile([PB, C1], bf16, tag="cumB")
                    if roll:
                        xs = xf[:, :, cs:cs + C1]
                        xk = [xs[:, k, :] for k in range(7)]
                    else:
                        gap = xg[:]
                        xk = [bass.AP(tensor=gap.tensor,
                                      offset=gap.offset + cs * 8 + k,
                                      ap=[gap.ap[0], [8, C1]])
                              for k in range(7)]
                    nc.vector.tensor_copy(xk[0], ang(uu, 0))
                    nc.vector.tensor_tensor(xk[1], ang(uu, 1), ang(tt, 0),
                                            op=Op.mult)
                    nc.vector.tensor_tensor(cumA[:], ang(tt, 0), ang(tt, 1),
                                            op=Op.mult)
                    nc.vector.tensor_tensor(xk[2], ang(uu, 2), cumA[:],
                                            op=Op.mult)
                    nc.vector.tensor_tensor(cumB[:], cumA[:], ang(tt, 2),
                                            op=Op.mult)
                    nc.vector.tensor_tensor(xk[3], ang(uu, 3), cumB[:],
                                            op=Op.mult)
                    nc.vector.tensor_tensor(cumA[:], cumB[:], ang(tt, 3),
                                            op=Op.mult)
                    nc.vector.tensor_tensor(xk[4], ang(uu, 4), cumA[:],
                                            op=Op.mult)
                    nc.vector.tensor_tensor(cumB[:], cumA[:], ang(tt, 4),
                                            op=Op.mult)
                    nc.vector.tensor_tensor(xk[5], ang(uu, 5), cumB[:],
                                            op=Op.mult)
                    nc.vector.tensor_tensor(xk[6], cumB[:], ang(tt, 5),
                                            op=Op.mult)

                st1.close()
                if mode in ("dma", "stage1"):
                    continue

                # ======== stage 2: neighbor products ========
                st2 = contextlib.ExitStack()
                p_xp = st2.enter_context(tc.tile_pool(name="p_xp", bufs=1))
                p_m = st2.enter_context(tc.tile_pool(name="p_m", bufs=2))
                p_pq = st2.enter_context(tc.tile_pool(name="p_pq", bufs=2 if roll else 1))
                junk_pool = st2.enter_context(tc.tile_pool(name="junk", bufs=2))

                for ch in range(N2):
                    cs = ch * C2
                    if roll:
                        xs = xf[:, :, cs:cs + C2]
                    else:
                        gap = xg[:]
                        xs = None
                        xg_k = lambda k0, n, off=0: bass.AP(
                            tensor=gap.tensor,
                            offset=gap.offset + cs * 8 + k0,
                            ap=[gap.ap[0], [1, n], [8, C2]])

                    for d in (0, 1):
                        # x' view(s) for this dir
                        if roll and d == 0:
                            lo = cs + L
                            if lo + C2 <= S:
                                xp_ap = xf[:, :, lo:lo + C2]
                            else:
                                xp = p_xp.tile([PB, 7, C2], bf16, tag="xp")
                                mn = S - lo
                                nc.vector.tensor_copy(xp[:, :, 0:mn],
                                                      xf[:, :, lo:S])
                                nc.vector.tensor_copy(xp[:, :, mn:C2],
                                                      xf[:, :, 0:C2 - mn])
                                xp_ap = xp[:]
                        elif roll and d == 1:
                            xp = p_xp.tile([PB, 7, C2], bf16, tag="xp")
                            nrow = C2 // L
                            src = bass.AP(
                                tensor=xf.tensor, offset=xf[:].offset + cs + 1,
                                ap=[xf[:].ap[0], [S, 7], [L, nrow], [1, L - 1]])
                            dst = bass.AP(
                                tensor=xp.tensor, offset=xp[:].offset,
                                ap=[xp[:].ap[0], [C2, 7], [L, nrow], [1, L - 1]])
                            nc.gpsimd.tensor_copy(dst, src)
                            srcw = bass.AP(
                                tensor=xf.tensor, offset=xf[:].offset + cs,
                                ap=[xf[:].ap[0], [S, 7], [L, nrow]])
                            dstw = bass.AP(
                                tensor=xp.tensor, offset=xp[:].offset + L - 1,
                                ap=[xp[:].ap[0], [C2, 7], [L, nrow]])
                            nc.gpsimd.tensor_copy(dstw, srcw)
                            xp_ap = xp[:]
                        else:
                            xpg = p_xp.tile([PB, C2, 8], bf16, tag="xp")
                            nc.gpsimd.ap_gather(
                                xpg[:], xg[:],
                                idx_sb[d][:, cs // 16:(cs + C2) // 16],
                                channels=PB, num_elems=S, d=8, num_idxs=C2)
                            gp = xpg[:]
                            xp_k = lambda k0, n: bass.AP(
                                tensor=gp.tensor, offset=gp.offset + k0,
                                ap=[gp.ap[0], [1, n], [8, C2]])

                        # m_k = x_k * x'_k  (7 rows, one op)
                        m = p_m.tile([PB, 7, C2], bf16, tag="m")
                        pq = p_pq.tile([PB, 6, C2], bf16, tag="pq")
                        if roll:
                            nc.vector.tensor_tensor(m[:], xs, xp_ap, op=Op.mult)
                            nc.vector.tensor_tensor(
                                pq[:, 0:3, :], xs[:, 0:3, :],
                                xp_ap[:, 4:7, :], op=Op.mult)
                            nc.vector.tensor_tensor(
                                pq[:, 3:6, :], xs[:, 4:7, :],
                                xp_ap[:, 0:3, :], op=Op.mult)
                        else:
                            nc.vector.tensor_tensor(m[:], xg_k(0, 7),
                                                    xp_k(0, 7), op=Op.mult)
                            nc.vector.tensor_tensor(pq[:, 0:3, :], xg_k(0, 3),
                                                    xp_k(4, 3), op=Op.mult)
                            nc.vector.tensor_tensor(pq[:, 3:6, :], xg_k(4, 3),
                                                    xp_k(0, 3), op=Op.mult)

                        # wr = m0+m1+m2+m3-m4-m5-m6 via subtract-fold, into m[0]
                        nc.vector.tensor_tensor(m[:, 0:3, :], m[:, 0:3, :],
                                                m[:, 4:7, :], op=Op.subtract)
                        nc.vector.tensor_tensor(m[:, 0:2, :], m[:, 0:2, :],
                                                m[:, 2:4, :], op=Op.add)
                        nc.vector.tensor_tensor(m[:, 0, :], m[:, 0, :],
                                                m[:, 1, :], op=Op.add)
                        # wi = sum(pq rows), into m[1]
                        nc.vector.tensor_tensor(pq[:, 0:3, :], pq[:, 0:3, :],
                                                pq[:, 3:6, :], op=Op.add)
                        nc.vector.tensor_tensor(pq[:, 0, :], pq[:, 0, :],
                                                pq[:, 1, :], op=Op.add)
                        nc.vector.tensor_tensor(m[:, 1, :], pq[:, 0, :],
                                                pq[:, 2, :], op=Op.add)

                        # acc += sum_s wr^2 + wi^2, one fused square over 2 rows
                        ia = ch * 2 + d
                        j1 = junk_pool.tile([PB, 2, C2], bf16, tag="junk")
                        nc.scalar.activation(j1[:], m[:, 0:2, :], Act.Square,
                                             accum_out=acc[:, ia:ia + 1])

                st2.close()

            # ======== final reduce + affine ========
            stot = small_pool.tile([PB, 1], f32)
            nc.vector.tensor_reduce(stot[:], acc[:], axis=X, op=Op.add)
            res = small_pool.tile([PB, 1], f32)
            nc.vector.tensor_scalar(res[:], stot[:], -NBETA, NBETA * 2.0 * S,
                                    op0=Op.mult, op1=Op.add)
            nc.sync.dma_start(out_d[:], res[:])

    nc.finalize()
    return nc


def kernel(phi, shift):
    from concourse.bass_utils import run_bass_kernel_spmd

    phi = np.ascontiguousarray(np.asarray(phi, dtype=np.float32))
    shift = np.asarray(shift, dtype=np.int32)
    key = (shift.tobytes(), 1)
    if key not in _cache:
        _cache[key] = _build(shift)
    nc = _cache[key]

    in_maps = [{"phi": phi[i * PB:(i + 1) * PB]} for i in range(NCORES)]
    res = run_bass_kernel_spmd(nc, in_maps, core_ids=list(range(NCORES)))
    out = np.concatenate([r["out"] for r in res.results], axis=0)
    return out.astype(np.float32)


# revision 14
# speedup vs baseline: 2.5307x; 1.3477x over previous
"""CP(n) lattice action kernel for Trainium2 (8 NeuronCores, Bass/Tile).

Math (matches reference):
  phi: (B=1024, S=4096, n=6) angles; shift: (2, S) neighbor site indices.
  Wrap: first 5 angles mod pi, last mod 2pi.
  x = hyperspherical embedding (7 comps); z = (x0..x3) + i(x4,x5,x6,0).
  w_d(s) = sum_k z_k(s) z_k(shift[d,s])
  action[b] = -4 * sum_{d,s} (|w_d(s)|^2 - 1)

Implementation notes:
  - Pure data parallel: batch axis sharded 8 x 128; batch on SBUF partitions,
    sites along the free dimension.
  - This platform is per-instruction-overhead dominated, so the kernel is
    built from few, large, multi-row instructions.
  - Angle wrap via identities (no floating mod on TRN2):
      phir = phi - 2pi*round(phi/2pi)  (round via fp->int RNE convert)
      sigma = Sign(phir)
      s_j  = sigma*Sin(phir) (j<5);  s_5 = Sin(phir)
      c_j  = Sin(sigma*pi/2 - phir) = sigma*cos(phir) (j<5); c_5 = sigma*that
  - Neighbor gather: shift values are read on host at build time; the
    roll-structured shift (nearest neighbor on a 64x64 lattice) lowers to
    offset/strided access patterns; arbitrary shift falls back to per-run
    copies.
  - Per-site |w|^2 and the site reduction fuse into ACT Square + accum_out.
"""
import contextlib
import sys

import numpy as np

sys.path.insert(0, "/opt/trn_rl_repo")

B, S, NA = 1024, 4096, 6
NCORES = 8
PB = B // NCORES          # 128 batches per core
C1 = 2048                 # stage-1 site chunk
N1 = S // C1
C2 = 2048                 # stage-2 site chunk
N2 = S // C2
L = 64                    # lattice row length
PI = float(np.pi)
NBETA = 4.0               # N * BETA

_cache = {}


def _detect_roll(shift):
    idx = np.arange(S).reshape(L, L)
    s0 = np.roll(idx, -1, axis=0).ravel()
    s1 = np.roll(idx, -1, axis=1).ravel()
    return np.array_equal(shift[0], s0) and np.array_equal(shift[1], s1)


def _runs(perm):
    runs = []
    st = 0
    for i in range(1, len(perm) + 1):
        if i == len(perm) or perm[i] != perm[i - 1] + 1:
            runs.append((st, int(perm[st]), i - st))
            st = i
    return runs


def _build(shift, reps=1, mode="full"):
    import concourse.bass as bass
    import concourse.tile as tile
    from concourse import bacc, mybir

    f32 = mybir.dt.float32
    bf16 = mybir.dt.bfloat16
    i16 = mybir.dt.int16
    Act = mybir.ActivationFunctionType
    Op = mybir.AluOpType
    X = mybir.AxisListType.X

    roll = _detect_roll(shift)

    nc = bacc.Bacc(None, target_bir_lowering=False)
    phi_d = nc.dram_tensor("phi", [PB, S, NA], f32, kind="ExternalInput")
    out_d = nc.dram_tensor("out", [PB, 1], f32, kind="ExternalOutput")
    pd_flat = phi_d[:].rearrange("p s a -> p (s a)")

    with tile.TileContext(nc) as tc:
        with contextlib.ExitStack() as ctx:
            xfull_pool = ctx.enter_context(tc.tile_pool(name="xfull", bufs=1))
            small_pool = ctx.enter_context(tc.tile_pool(name="small", bufs=1))

            NACC = N2
            acc = small_pool.tile([PB, NACC], f32)
            if roll:
                xf = xfull_pool.tile([PB, 7, S], bf16)
                xg = None
            else:
                # site-major cells [site, 8] so gpsimd.ap_gather can fetch
                # whole 7-component cells per shift index
                xf = None
                xg = xfull_pool.tile([PB, S, 8], bf16)
                idx_sb = []
                for d in range(2):
                    wrapped = np.zeros((PB, S // 16), np.int16)
                    base = shift[d].reshape(S // 16, 16).T.astype(np.int16)
                    for g in range(PB // 16):
                        wrapped[16 * g:16 * (g + 1)] = base
                    hdl = nc.inline_tensor(wrapped, name=f"shift_idx_{d}")
                    t_ = small_pool.tile([PB, S // 16], mybir.dt.int16, tag=f"idx{d}")
                    nc.sync.dma_start(t_[:], hdl[:])
                    idx_sb.append(t_)

            for rep in range(reps):
                # ======== stage 1: wrap + trig + embedding ========
                st1 = contextlib.ExitStack()
                p_phi = st1.enter_context(tc.tile_pool(name="p_phi", bufs=1))
                p_ks = st1.enter_context(tc.tile_pool(name="p_ks", bufs=1))
                p_t = st1.enter_context(tc.tile_pool(name="p_t", bufs=1))
                p_u = st1.enter_context(tc.tile_pool(name="p_u", bufs=1))
                p_cum = st1.enter_context(tc.tile_pool(name="p_cum", bufs=1))

                for ch in range(N1):
                    cs = ch * C1
                    M = C1 * NA

                    phic = p_phi.tile([PB, M], f32, tag="phic")
                    nc.sync.dma_start(phic[:], pd_flat[:, cs * NA:(cs + C1) * NA])

                    if mode == "dma":
                        nc.vector.tensor_reduce(acc[:, 0:1], phic[:, 0:8],
                                                axis=X, op=Op.add)
                        continue

                    # k = round(phi/2pi) as int16
                    k = p_ks.tile([PB, M], i16, tag="ks")
                    nc.vector.tensor_scalar(k[:], phic[:], 1.0 / (2 * PI), None,
                                            op0=Op.mult)
                    # phir = (k * -2pi) + phi   (in place)
                    nc.vector.scalar_tensor_tensor(
                        phic[:], k[:], -2 * PI, phic[:], op0=Op.mult, op1=Op.add)

                    # sigma, t = Sin(phir)  (interleaved site-major, bf16)
                    sig = p_ks.tile([PB, M], bf16, tag="ks")
                    nc.scalar.activation(sig[:], phic[:], Act.Sign)
                    tt = p_t.tile([PB, M], bf16, tag="t")
                    nc.scalar.activation(tt[:], phic[:], Act.Sin)
                    # arg2 = sigma*pi/2 - phir (in place over phir)
                    nc.vector.scalar_tensor_tensor(
                        phic[:], sig[:], PI / 2, phic[:],
                        op0=Op.mult, op1=Op.subtract)
                    # u = Sin(arg2) = sigma*cos(phir)
                    uu = p_u.tile([PB, M], bf16, tag="u")
                    nc.scalar.activation(uu[:], phic[:], Act.Sin)

                    def ang(tile_, j, n=1):
                        ap = tile_[:]
                        if n == 1:
                            return bass.AP(tensor=ap.tensor, offset=ap.offset + j,
                                           ap=[ap.ap[0], [NA, C1]])
                        return bass.AP(tensor=ap.tensor, offset=ap.offset + j,
                                       ap=[ap.ap[0], [NA, C1], [1, n]])

                    # s_j = sigma*t for j<5 (in place on t)
                    nc.vector.tensor_tensor(ang(tt, 0, 5), ang(tt, 0, 5),
                                            ang(sig, 0, 5), op=Op.mult)
                    # c_5 = sigma*u at j=5 (in place on u)
                    nc.vector.tensor_tensor(ang(uu, 5), ang(uu, 5),
                                            ang(sig, 5), op=Op.mult)

                    # cumprod + x build into xf rows / xg cells
                    cumA = p_cum.tile([PB, C1], bf16, tag="cumA")
                    cumB = p_cum.tile([PB, C1], bf16, tag="cumB")
                    if roll:
                        xs = xf[:, :, cs:cs + C1]
                        xk = [xs[:, k, :] for k in range(7)]
                    else:
                        gap = xg[:]
                        xk = [bass.AP(tensor=gap.tensor,
                                      offset=gap.offset + cs * 8 + k,
                                      ap=[gap.ap[0], [8, C1]])
                              for k in range(7)]
                    nc.vector.tensor_copy(xk[0], ang(uu, 0))
                    nc.vector.tensor_tensor(xk[1], ang(uu, 1), ang(tt, 0),
                                            op=Op.mult)
                    nc.vector.tensor_tensor(cumA[:], ang(tt, 0), ang(tt, 1),
                                            op=Op.mult)
                    nc.vector.tensor_tensor(xk[2], ang(uu, 2), cumA[:],
                                            op=Op.mult)
                    nc.vector.tensor_tensor(cumB[:], cumA[:], ang(tt, 2),
                                            op=Op.mult)
                    nc.vector.tensor_tensor(xk[3], ang(uu, 3), cumB[:],
                                            op=Op.mult)
                    nc.vector.tensor_tensor(cumA[:], cumB[:], ang(tt, 3),
                                            op=Op.mult)
                    nc.vector.tensor_tensor(xk[4], ang(uu, 4), cumA[:],
                                            op=Op.mult)
                    nc.vector.tensor_tensor(cumB[:], cumA[:], ang(tt, 4),
                                            op=Op.mult)
                    nc.vector.tensor_tensor(xk[5], ang(uu, 5), cumB[:],
                                            op=Op.mult)
                    nc.vector.tensor_tensor(xk[6], cumB[:], ang(tt, 5),
                                            op=Op.mult)

                st1.close()
                if mode in ("dma", "stage1"):
                    continue

                # ======== stage 2: neighbor products ========
                st2 = contextlib.ExitStack()
                p_xp = st2.enter_context(tc.tile_pool(name="p_xp", bufs=1))
                p_m = st2.enter_context(tc.tile_pool(name="p_m", bufs=1))
                p_pq = st2.enter_context(tc.tile_pool(name="p_pq", bufs=1))
                junk_pool = st2.enter_context(tc.tile_pool(name="junk", bufs=1))

                for ch in range(N2):
                    cs = ch * C2
                    if roll:
                        xs = xf[:, :, cs:cs + C2]
                    else:
                        gap = xg[:]
                        xs = None
                        xg_k = lambda k0, n, off=0: bass.AP(
                            tensor=gap.tensor,
                            offset=gap.offset + cs * 8 + k0,
                            ap=[gap.ap[0], [1, n], [8, C2]])

                    # double-width: both dirs side by side, shared folds
                    m = p_m.tile([PB, 7, 2 * C2], bf16, tag="m")
                    pq = p_pq.tile([PB, 6, 2 * C2], bf16, tag="pq")

                    for d in (0, 1):
                        if roll and d == 0:
                            lo = cs + L
                            if lo + C2 <= S:
                                xp_ap = xf[:, :, lo:lo + C2]
                            else:
                                xp = p_xp.tile([PB, 7, C2], bf16, tag="xp")
                                mn = S - lo
                                nc.vector.tensor_copy(xp[:, :, 0:mn],
                                                      xf[:, :, lo:S])
                                nc.vector.tensor_copy(xp[:, :, mn:C2],
                                                      xf[:, :, 0:C2 - mn])
                                xp_ap = xp[:]
                        elif roll and d == 1:
                            xp = p_xp.tile([PB, 7, C2], bf16, tag="xp")
                            nrow = C2 // L
                            src = bass.AP(
                                tensor=xf.tensor, offset=xf[:].offset + cs + 1,
                                ap=[xf[:].ap[0], [S, 7], [L, nrow], [1, L - 1]])
                            dst = bass.AP(
                                tensor=xp.tensor, offset=xp[:].offset,
                                ap=[xp[:].ap[0], [C2, 7], [L, nrow], [1, L - 1]])
                            nc.gpsimd.tensor_copy(dst, src)
                            srcw = bass.AP(
                                tensor=xf.tensor, offset=xf[:].offset + cs,
                                ap=[xf[:].ap[0], [S, 7], [L, nrow]])
                            dstw = bass.AP(
                                tensor=xp.tensor, offset=xp[:].offset + L - 1,
                                ap=[xp[:].ap[0], [C2, 7], [L, nrow]])
                            nc.gpsimd.tensor_copy(dstw, srcw)
                            xp_ap = xp[:]
                        else:
                            xpg = p_xp.tile([PB, C2, 8], bf16, tag="xp")
                            nc.gpsimd.ap_gather(
                                xpg[:], xg[:],
                                idx_sb[d][:, cs // 16:(cs + C2) // 16],
                                channels=PB, num_elems=S, d=8, num_idxs=C2)
                            gp = xpg[:]
                            xp_k = lambda k0, n: bass.AP(
                                tensor=gp.tensor, offset=gp.offset + k0,
                                ap=[gp.ap[0], [1, n], [8, C2]])

                        ms = m[:, :, d * C2:(d + 1) * C2]
                        pqs = pq[:, :, d * C2:(d + 1) * C2]
                        if roll:
                            nc.vector.tensor_tensor(ms, xs, xp_ap, op=Op.mult)
                            nc.vector.tensor_tensor(
                                pqs[:, 0:3, :], xs[:, 0:3, :],
                                xp_ap[:, 4:7, :], op=Op.mult)
                            nc.vector.tensor_tensor(
                                pqs[:, 3:6, :], xs[:, 4:7, :],
                                xp_ap[:, 0:3, :], op=Op.mult)
                        else:
                            nc.vector.tensor_tensor(ms, xg_k(0, 7),
                                                    xp_k(0, 7), op=Op.mult)
                            nc.vector.tensor_tensor(pqs[:, 0:3, :], xg_k(0, 3),
                                                    xp_k(4, 3), op=Op.mult)
                            nc.vector.tensor_tensor(pqs[:, 3:6, :], xg_k(4, 3),
                                                    xp_k(0, 3), op=Op.mult)

                    # shared folds across both dirs (double width)
                    nc.vector.tensor_tensor(m[:, 0:3, :], m[:, 0:3, :],
                                            m[:, 4:7, :], op=Op.subtract)
                    nc.vector.tensor_tensor(m[:, 0:2, :], m[:, 0:2, :],
                                            m[:, 2:4, :], op=Op.add)
                    nc.vector.tensor_tensor(m[:, 0, :], m[:, 0, :],
                                            m[:, 1, :], op=Op.add)
                    nc.vector.tensor_tensor(pq[:, 0:3, :], pq[:, 0:3, :],
                                            pq[:, 3:6, :], op=Op.add)
                    nc.vector.tensor_tensor(pq[:, 0, :], pq[:, 0, :],
                                            pq[:, 1, :], op=Op.add)
                    nc.vector.tensor_tensor(m[:, 1, :], pq[:, 0, :],
                                            pq[:, 2, :], op=Op.add)

                    # one fused square+accum per chunk (wr,wi x both dirs)
                    j1 = junk_pool.tile([PB, 2, 2 * C2], bf16, tag="junk")
                    nc.scalar.activation(j1[:], m[:, 0:2, :], Act.Square,
                                         accum_out=acc[:, ch:ch + 1])

                st2.close()

            # ======== final reduce + affine ========
            stot = small_pool.tile([PB, 1], f32)
            nc.vector.tensor_reduce(stot[:], acc[:], axis=X, op=Op.add)
            res = small_pool.tile([PB, 1], f32)
            nc.vector.tensor_scalar(res[:], stot[:], -NBETA, NBETA * 2.0 * S,
                                    op0=Op.mult, op1=Op.add)
            nc.sync.dma_start(out_d[:], res[:])

    nc.finalize()
    return nc


def kernel(phi, shift):
    from concourse.bass_utils import run_bass_kernel_spmd

    phi = np.ascontiguousarray(np.asarray(phi, dtype=np.float32))
    shift = np.asarray(shift, dtype=np.int32)
    key = (shift.tobytes(), 1)
    if key not in _cache:
        _cache[key] = _build(shift)
    nc = _cache[key]

    in_maps = [{"phi": phi[i * PB:(i + 1) * PB]} for i in range(NCORES)]
    res = run_bass_kernel_spmd(nc, in_maps, core_ids=list(range(NCORES)))
    out = np.concatenate([r["out"] for r in res.results], axis=0)
    return out.astype(np.float32)
